# revision 3
# baseline (speedup 1.0000x reference)
"""Trainium2 Bass kernel for the Krylov/SSM problem.

K[h, l] = C[h] @ dA[h]^l @ dB[h],  l = 0..L-1
  dA = (I - (dt/2)A)^-1 (I + (dt/2)A),  dB = dt * (I - (dt/2)A)^-1 B

Device algorithm (per channel, fp32 throughout):
  E = (dt/2) A           (host prep, packed 2 channels per 128x128 block-diag tile)
  Neumann-product inverse: G = (I+E)(I+E^2)(I+E^4)(I+E^8)   [rho(E) <= ~0.2]
  dA  = (I+E)^2 (I+E^2)(I+E^4)(I+E^8)
  Ct  = G^T C            (solve folded into the C side; G commutes with dA)
  V   = [dt*B, dA(dt*B), ..., dA^63(dt*B)]        (doubling)
  U   = [Ct, M^T Ct, ..., (M^T)^63 Ct], M = dA^64 (doubling, powers to dA^2048)
  K[h, 64j + i] = (U^T V)[j, i]

All matmuls run as 128x128 block-diagonal ops (2 channels / PE pass). The
matmul primitive computes lhsT.T @ rhs, so the squaring chain keeps both
X and X^T for every power ("pair squaring").

Sharding: H axis, 32 channels (16 tiles) per core across 8 cores. SPMD, no
communication.
"""

import numpy as np

H, N, L = 256, 64, 4096
NCORES = 8
CH_PER_CORE = H // NCORES   # 32
NT = CH_PER_CORE // 2       # 16 block-diag tiles per core

_cache = {}


def _build_program(nt=NT):
    import concourse.bacc as bacc
    import concourse.tile as tile
    import concourse.mybir as mybir

    f32 = mybir.dt.float32
    nc = bacc.Bacc("TRN2", target_bir_lowering=False, debug=False)

    Ein = nc.dram_tensor("Ein", [nt, 128, 128], f32, kind="ExternalInput").ap()
    ETin = nc.dram_tensor("ETin", [nt, 128, 128], f32, kind="ExternalInput").ap()
    F0in = nc.dram_tensor("F0in", [nt, 128, 128], f32, kind="ExternalInput").ap()
    F0Tin = nc.dram_tensor("F0Tin", [nt, 128, 128], f32, kind="ExternalInput").ap()
    BCin = nc.dram_tensor("BCin", [nt, 128, 2], f32, kind="ExternalInput").ap()
    Iin = nc.dram_tensor("Iin", [128, 128], f32, kind="ExternalInput").ap()
    Y = nc.dram_tensor("Y", [nt, 128, 64], f32, kind="ExternalOutput").ap()

    with tile.TileContext(nc) as tc:
        with (
            tc.tile_pool(name="const", bufs=1) as cpool,
            tc.tile_pool(name="sb", bufs=6) as sb,
            tc.tile_pool(name="ps", bufs=5, space="PSUM") as ps,
        ):
            ident = cpool.tile([128, 128], f32, name="ident")
            nc.sync.dma_start(ident[:], Iin)

            # alternate PSUM->SBUF copies between DVE and ACT
            state = {"i": 0}

            def copy_eng():
                state["i"] += 1
                return state["i"] % 2

            def CP(tag, p, w=128):
                s = sb.tile([128, w], f32, tag=tag, name=tag)
                if copy_eng():
                    nc.vector.tensor_copy(s[:], p[:])
                else:
                    nc.scalar.copy(s[:], p[:])
                return s

            for t in range(nt):
                def MM(tag, lhsT, rhs, w=128, slots="mm", nb=5):
                    p = ps.tile([128, w], f32, tag=slots, bufs=nb, name=tag + "_p")
                    nc.tensor.matmul(p[:], lhsT, rhs, start=True, stop=True)
                    return p

                def LOAD(tag, src, w=128):
                    s = sb.tile([128, w], f32, tag=tag, name=tag)
                    nc.sync.dma_start(s[:], src)
                    return s

                E = LOAD("E", Ein[t])
                ET = LOAD("ET", ETin[t])
                F0 = LOAD("F0", F0in[t])
                F0T = LOAD("F0T", F0Tin[t])
                BC = LOAD("BC", BCin[t], w=2)

                E2p = MM("E2", ET[:], E[:])
                E2 = CP("E2", E2p)
                F1 = sb.tile([128, 128], f32, tag="F1", name="F1")
                nc.vector.tensor_add(F1[:], E2p[:], ident[:])
                s0 = sb.tile([128, 128], f32, tag="s0", name="s0")
                nc.gpsimd.tensor_add(s0[:], F0[:], E[:])
                F0sq = sb.tile([128, 128], f32, tag="F0sq", name="F0sq")
                nc.vector.tensor_add(F0sq[:], E2p[:], s0[:])

                E2Tp = MM("E2T", E[:], ET[:])
                E2T = CP("E2T", E2Tp)
                s0T = sb.tile([128, 128], f32, tag="s0T", name="s0T")
                nc.gpsimd.tensor_add(s0T[:], F0T[:], ET[:])
                F0sqT = sb.tile([128, 128], f32, tag="F0sqT", name="F0sqT")
                nc.vector.tensor_add(F0sqT[:], E2Tp[:], s0T[:])

                E4p = MM("E4", E2T[:], E2[:])
                E4 = CP("E4", E4p)
                F2 = sb.tile([128, 128], f32, tag="F2", name="F2")
                nc.vector.tensor_add(F2[:], E4p[:], ident[:])
                E4Tp = MM("E4T", E2[:], E2T[:])
                E4T = CP("E4T", E4Tp)
                F2T = sb.tile([128, 128], f32, tag="F2T", name="F2T")
                nc.vector.tensor_add(F2T[:], E4Tp[:], ident[:])

                E8p = MM("E8", E4T[:], E4[:])
                F3 = sb.tile([128, 128], f32, tag="F3", name="F3")
                nc.vector.tensor_add(F3[:], E8p[:], ident[:])

                # PAT = (F0sq@F1)^T = F1^T F0sq^T ; PB = F2@F3
                PATp = MM("PAT", F1[:], F0sqT[:])
                PAT = CP("PAT", PATp)
                PBp = MM("PB", F2T[:], F3[:])
                PB = CP("PB", PBp)
                # dA = PA@PB ; dAT = PB^T PA^T
                N0p = MM("N0", PAT[:], PB[:])
                T0p = MM("T0", PB[:], PAT[:])
                Ns = {0: CP("N0", N0p)}
                Ts = {0: CP("T0", T0p)}

                # Ct = F3^T F2^T F1^T F0^T C   (width-1 matvec chain)
                c = BC[:, 1:2]
                for ci, Fk in enumerate((F0, F1, F2, F3)):
                    cp = MM(f"c{ci}", Fk[:], c, w=1, slots="sm", nb=2)
                    if ci < 3:
                        c = CP(f"c{ci}", cp, w=1)

                U = sb.tile([128, 64], f32, tag="U", name="U")
                V = sb.tile([128, 64], f32, tag="V", name="V")
                nc.scalar.copy(U[:, 0:1], cp[:])
                nc.scalar.copy(V[:, 0:1], BC[:, 0:1])

                # power chain (pair squaring) + V applies (T_k) interleaved
                for k in range(1, 12):
                    w = 1 << (k - 1)
                    if k <= 6:
                        vp = MM(f"va{k}", Ts[k - 1][:], V[:, 0:w], w=w,
                                slots="sm", nb=2)
                        if copy_eng():
                            nc.vector.tensor_copy(V[:, w:2 * w], vp[:])
                        else:
                            nc.scalar.copy(V[:, w:2 * w], vp[:])
                    Np = MM(f"N{k}", Ts[k - 1][:], Ns[k - 1][:])
                    Ns[k] = CP(f"N{k}", Np)
                    if k <= 10:
                        Tp = MM(f"T{k}", Ns[k - 1][:], Ts[k - 1][:])
                        Ts[k] = CP(f"T{k}", Tp)

                # U applies (N_{6+j})
                for j in range(6):
                    w = 1 << j
                    up = MM(f"ua{j}", Ns[6 + j][:], U[:, 0:w], w=w,
                            slots="sm", nb=2)
                    if copy_eng():
                        nc.vector.tensor_copy(U[:, w:2 * w], up[:])
                    else:
                        nc.scalar.copy(U[:, w:2 * w], up[:])

                # K2d per channel: Ka = U_a^T V_a, Kb = U_b^T V_b
                Kap = ps.tile([64, 64], f32, tag="sm", bufs=2, name="Kap")
                nc.tensor.matmul(Kap[:], U[0:64, :], V[0:64, :],
                                 start=True, stop=True)
                Ka = sb.tile([64, 64], f32, tag="Ka", name="Ka")
                nc.scalar.copy(Ka[:], Kap[:])
                nc.sync.dma_start(Y[t, 0:64, :], Ka[:])
                Kbp = ps.tile([64, 64], f32, tag="sm", bufs=2, name="Kbp")
                nc.tensor.matmul(Kbp[:], U[64:128, :], V[64:128, :],
                                 start=True, stop=True)
                Kb = sb.tile([64, 64], f32, tag="Kb", name="Kb")
                nc.vector.tensor_copy(Kb[:], Kbp[:])
                nc.sync.dma_start(Y[t, 64:128, :], Kb[:])

    nc.compile()
    return nc


def _host_pack(A, B, C, log_dt):
    A = np.asarray(A, np.float32)
    B = np.asarray(B, np.float32)
    C = np.asarray(C, np.float32)
    log_dt = np.asarray(log_dt, np.float32)
    dt = np.exp(log_dt)
    E = (0.5 * dt)[:, None, None].astype(np.float32) * A      # [H,64,64]
    ETc = np.ascontiguousarray(np.swapaxes(E, 1, 2))
    dtB = (dt[:, None] * B).astype(np.float32)

    G = A.shape[0] // 2
    Epk = np.zeros((G, 128, 128), np.float32)
    ETpk = np.zeros((G, 128, 128), np.float32)
    Epk[:, 0:64, 0:64] = E[0::2]
    Epk[:, 64:128, 64:128] = E[1::2]
    ETpk[:, 0:64, 0:64] = ETc[0::2]
    ETpk[:, 64:128, 64:128] = ETc[1::2]
    I128 = np.eye(128, dtype=np.float32)
    F0pk = Epk + I128
    F0Tpk = ETpk + I128
    BCpk = np.zeros((G, 128, 2), np.float32)
    BCpk[:, 0:64, 0] = dtB[0::2]
    BCpk[:, 64:128, 0] = dtB[1::2]
    BCpk[:, 0:64, 1] = C[0::2]
    BCpk[:, 64:128, 1] = C[1::2]
    return Epk, ETpk, F0pk, F0Tpk, BCpk, I128


def kernel(A, B, C, log_dt, L):
    from concourse.bass_utils import run_bass_kernel_spmd

    if "nc" not in _cache:
        _cache["nc"] = _build_program(NT)
    nc = _cache["nc"]

    Epk, ETpk, F0pk, F0Tpk, BCpk, I128 = _host_pack(A, B, C, log_dt)
    in_maps = []
    for c in range(NCORES):
        s = slice(c * NT, (c + 1) * NT)
        in_maps.append({
            "Ein": Epk[s], "ETin": ETpk[s], "F0in": F0pk[s],
            "F0Tin": F0Tpk[s], "BCin": BCpk[s], "Iin": I128,
        })
    res = run_bass_kernel_spmd(nc, in_maps, core_ids=list(range(NCORES)))
    K = np.empty((H, L), np.float32)
    for c in range(NCORES):
        K[c * CH_PER_CORE:(c + 1) * CH_PER_CORE] = (
            res.results[c]["Y"].reshape(CH_PER_CORE, L))
    return K


# revision 14
# speedup vs baseline: 4428.3700x; 4428.3700x over previous
"""Trainium2 Bass kernel for the Krylov/SSM problem.

K[h, l] = C[h] @ dA[h]^l @ dB[h],  l = 0..L-1
  dA = (I - (dt/2)A)^-1 (I + (dt/2)A),  dB = dt * (I - (dt/2)A)^-1 B

Device algorithm (per channel, fp32 throughout):
  E = (dt/2) A           (host prep, packed 2 channels per 128x128 block-diag tile)
  Neumann-product inverse: G = (I+E)(I+E^2)(I+E^4)(I+E^8)   [rho(E) <= ~0.2]
  dA  = (I+E)^2 (I+E^2)(I+E^4)(I+E^8)
  Ct  = G^T C            (solve folded into the C side; G commutes with dA)
  V   = [dt*B, dA(dt*B), ..., dA^63(dt*B)]        (doubling)
  U   = [Ct, M^T Ct, ..., (M^T)^63 Ct], M = dA^64 (doubling, powers to dA^2048)
  K[h, 64j + i] = (U^T V)[j, i]

All matmuls run as 128x128 block-diagonal ops (2 channels / PE pass). The
matmul primitive computes lhsT.T @ rhs, so the squaring chain keeps both
X and X^T for every power ("pair squaring"); each level's (N, T) pair lands
in one PSUM bank and moves to SBUF with a single [128,256] copy.

Instructions are emitted LEVEL-MAJOR across the 16 per-core tiles so each
engine's (in-order) instruction stream always has ready work from other
tiles while one tile's dependencies drain.

Sharding: H axis, 32 channels (16 tiles) per core across 8 cores. SPMD, no
communication.
"""

import numpy as np

H, N, L = 256, 64, 4096
NCORES = 8
CH_PER_CORE = H // NCORES   # 32
NT = CH_PER_CORE // 2       # 16 block-diag tiles per core

_cache = {}


def _build_program(nt=NT, repeat=None):
    import contextlib
    import concourse.bacc as bacc
    import concourse.tile as tile
    import concourse.mybir as mybir

    f32 = mybir.dt.float32
    nc = bacc.Bacc("TRN2", target_bir_lowering=False, debug=False)

    Ein = nc.dram_tensor("Ein", [nt, 128, 128], f32, kind="ExternalInput").ap()
    ETin = nc.dram_tensor("ETin", [nt, 128, 128], f32, kind="ExternalInput").ap()
    H0in = nc.dram_tensor("H0in", [nt, 128, 128], f32, kind="ExternalInput").ap()
    H0Tin = nc.dram_tensor("H0Tin", [nt, 128, 128], f32, kind="ExternalInput").ap()
    BCin = nc.dram_tensor("BCin", [nt, 128, 2], f32, kind="ExternalInput").ap()
    Iin = nc.dram_tensor("Iin", [128, 128], f32, kind="ExternalInput").ap()
    Y = nc.dram_tensor("Y", [nt, 128, 64], f32, kind="ExternalOutput").ap()

    with tile.TileContext(nc) as tc:
        with (
            tc.tile_pool(name="const", bufs=1) as cpool,
            tc.tile_pool(name="sb", bufs=1) as sb,
            tc.tile_pool(name="ps", bufs=1, space="PSUM") as ps,
        ):
            ident = cpool.tile([128, 128], f32, name="ident")
            nc.sync.dma_start(ident[:], Iin)
            rep = tc.For_i(0, repeat, 1) if repeat else contextlib.nullcontext()
            rep.__enter__()

            state = {"i": 0}

            def ve():  # 2:1 DVE:ACT split for PSUM->SBUF traffic
                state["i"] += 1
                return state["i"] % 9 < 5

            def SBT(tag, w=128, bufs=nt + 1):
                return sb.tile([128, w], f32, tag=tag, name=tag, bufs=bufs)

            def PW(w=256):
                """One PSUM bank (pair = [N | T] side by side when w=256)."""
                return ps.tile([128, w], f32, tag="mm", bufs=6, name="pw")

            def MM(out_ap, lhsT, rhs):
                nc.tensor.matmul(out_ap, lhsT, rhs, start=True, stop=True)

            def CPW(tag, p, w=256, bufs=None):
                # all wide stage tensors share one ring: ~2 levels x nt live
                s = sb.tile([128, w], f32, tag="ring", name=tag,
                            bufs=2 * nt + 4)
                if ve():
                    nc.vector.tensor_copy(s[:], p[:, 0:w])
                else:
                    nc.scalar.copy(s[:], p[:, 0:w])
                return s

            def SMM(lhsT, rhs, w):
                p = ps.tile([128, w], f32, tag="sm", bufs=2, name="sp")
                nc.tensor.matmul(p[:], lhsT, rhs, start=True, stop=True)
                return p

            T = [dict() for _ in range(nt)]
            tiles = range(nt)

            def s_load(t):
                d = T[t]
                dma = [nc.sync, nc.gpsimd][t % 2]
                for nm, srcap in (("E", Ein), ("ET", ETin), ("H0", H0in),
                                  ("H0T", H0Tin)):
                    d[nm] = SBT(nm, bufs=8)
                    dma.dma_start(d[nm][:], srcap[t])
                d["BC"] = SBT("BC", w=2)
                dma.dma_start(d["BC"][:], BCin[t])

            def s_f0(t):
                d = T[t]
                d["F0"] = SBT("F0", bufs=12)
                nc.gpsimd.tensor_add(d["F0"][:], d["E"][:], ident[:])
                d["F0T"] = SBT("F0T", bufs=8)
                nc.gpsimd.tensor_add(d["F0T"][:], d["ET"][:], ident[:])

            def s_e2(t):
                d = T[t]
                p = PW()
                MM(p[:, 0:128], d["ET"][:], d["E"][:])
                MM(p[:, 128:256], d["E"][:], d["ET"][:])
                d["E2NT"] = CPW("E2NT", p)
                d["F1"] = SBT("F1", bufs=12)
                nc.gpsimd.tensor_add(d["F1"][:], d["E2NT"][:, 0:128], ident[:])
                d["F0sq"] = SBT("F0sq", bufs=8)
                nc.vector.tensor_add(d["F0sq"][:], p[:, 0:128], d["H0"][:])
                d["F0sqT"] = SBT("F0sqT", bufs=8)
                nc.vector.tensor_add(d["F0sqT"][:], p[:, 128:256], d["H0T"][:])

            def s_e4(t):
                d = T[t]
                p = PW()
                MM(p[:, 0:128], d["E2NT"][:, 128:256], d["E2NT"][:, 0:128])
                MM(p[:, 128:256], d["E2NT"][:, 0:128], d["E2NT"][:, 128:256])
                d["E4NT"] = CPW("E4NT", p)
                d["F2"] = SBT("F2", bufs=12)
                nc.gpsimd.tensor_add(d["F2"][:], d["E4NT"][:, 0:128], ident[:])
                d["F2T"] = SBT("F2T", bufs=8)
                nc.gpsimd.tensor_add(d["F2T"][:], d["E4NT"][:, 128:256], ident[:])

            def s_e8(t):
                d = T[t]
                p = SMM(d["E4NT"][:, 128:256], d["E4NT"][:, 0:128], w=128)
                d["F3"] = SBT("F3", bufs=12)
                nc.vector.tensor_add(d["F3"][:], p[:], ident[:])

            def s_pp(t):
                d = T[t]
                p = PW()
                MM(p[:, 0:128], d["F1"][:], d["F0sqT"][:])
                MM(p[:, 128:256], d["F2T"][:], d["F3"][:])
                d["PP"] = CPW("PP", p)

            def s_nt0(t):
                d = T[t]
                PAT, PB = d["PP"][:, 0:128], d["PP"][:, 128:256]
                p = PW()
                MM(p[:, 0:128], PAT, PB)
                MM(p[:, 128:256], PB, PAT)
                d["NT0"] = CPW("NT0", p)

            def mk_c(ci):
                def s_c(t):
                    d = T[t]
                    if ci == 0:
                        d["U"] = SBT("U", w=64, bufs=nt + 1)
                        d["V"] = SBT("V", w=64, bufs=nt + 1)
                        nc.scalar.copy(d["V"][:, 0:1], d["BC"][:, 0:1])
                    Fk = d[("F0", "F1", "F2", "F3")[ci]]
                    c = d["BC"][:, 1:2] if ci == 0 else d[f"c{ci-1}"][:]
                    cp = SMM(Fk[:], c, w=1)
                    if ci < 3:
                        cs = SBT(f"c{ci}", w=1, bufs=8)
                        nc.scalar.copy(cs[:], cp[:])
                        d[f"c{ci}"] = cs
                    else:
                        nc.scalar.copy(d["U"][:, 0:1], cp[:])
                return s_c

            def mk_pow(k):
                def s_pow(t):
                    d = T[t]
                    if k <= 11:
                        Nk1 = d[f"NT{k-1}"][:, 0:128]
                        Tk1 = d[f"NT{k-1}"][:, 128:256]
                    if k <= 6:
                        w = 1 << (k - 1)
                        vp = SMM(Tk1, d["V"][:, 0:w], w=w)
                        if ve():
                            nc.vector.tensor_copy(d["V"][:, w:2 * w], vp[:])
                        else:
                            nc.scalar.copy(d["V"][:, w:2 * w], vp[:])
                    if k >= 7:
                        j = k - 7          # U-apply j reads NT_{6+j} = NT_{k-1}
                        w = 1 << j
                        up = SMM(d[f"NT{6+j}"][:, 0:128], d["U"][:, 0:w], w=w)
                        if ve():
                            nc.vector.tensor_copy(d["U"][:, w:2 * w], up[:])
                        else:
                            nc.scalar.copy(d["U"][:, w:2 * w], up[:])
                    if k <= 10:
                        p = PW()
                        MM(p[:, 0:128], Tk1, Nk1)
                        MM(p[:, 128:256], Nk1, Tk1)
                        d[f"NT{k}"] = CPW(f"NT{k}", p)
                    elif k == 11:
                        p = SMM(Tk1, Nk1, w=128)
                        d[f"NT{k}"] = CPW(f"NT{k}", p, w=128)
                return s_pow

            def s_fin(t):
                d = T[t]
                Kap = ps.tile([64, 64], f32, tag="sm", bufs=2, name="Kap")
                nc.tensor.matmul(Kap[:], d["U"][0:64, :], d["V"][0:64, :],
                                 start=True, stop=True)
                Ka = sb.tile([64, 64], f32, tag="Ka", name="Ka", bufs=4)
                nc.scalar.copy(Ka[:], Kap[:])
                nc.scalar.dma_start(Y[t, 0:64, :], Ka[:])
                Kbp = ps.tile([64, 64], f32, tag="sm", bufs=2, name="Kbp")
                nc.tensor.matmul(Kbp[:], d["U"][64:128, :], d["V"][64:128, :],
                                 start=True, stop=True)
                Kb = sb.tile([64, 64], f32, tag="Kb", name="Kb", bufs=4)
                nc.vector.tensor_copy(Kb[:], Kbp[:])
                nc.sync.dma_start(Y[t, 64:128, :], Kb[:])

            stages = ([s_load, s_f0, s_e2, s_e4, s_e8] +
                      [mk_c(0), mk_c(1), mk_c(2), mk_c(3)] +
                      [s_pp, s_nt0] +
                      [mk_pow(k) for k in range(1, 13)] +
                      [s_fin])
            ns = len(stages)
            # skewed (wavefront) emission: tile t runs stage s at step s + t
            for step in range(ns + nt - 1):
                for t in tiles:
                    s = step - t
                    if 0 <= s < ns:
                        stages[s](t)
            rep.__exit__(None, None, None)

    nc.compile()
    return nc


def _host_pack(A, B, C, log_dt):
    A = np.asarray(A, np.float32)
    B = np.asarray(B, np.float32)
    C = np.asarray(C, np.float32)
    log_dt = np.asarray(log_dt, np.float32)
    dt = np.exp(log_dt)
    E = (0.5 * dt)[:, None, None].astype(np.float32) * A      # [H,64,64]
    ETc = np.ascontiguousarray(np.swapaxes(E, 1, 2))
    dtB = (dt[:, None] * B).astype(np.float32)

    G = A.shape[0] // 2
    Epk = np.zeros((G, 128, 128), np.float32)
    ETpk = np.zeros((G, 128, 128), np.float32)
    Epk[:, 0:64, 0:64] = E[0::2]
    Epk[:, 64:128, 64:128] = E[1::2]
    ETpk[:, 0:64, 0:64] = ETc[0::2]
    ETpk[:, 64:128, 64:128] = ETc[1::2]
    I128 = np.eye(128, dtype=np.float32)
    H0pk = 2.0 * Epk + I128
    H0Tpk = 2.0 * ETpk + I128
    BCpk = np.zeros((G, 128, 2), np.float32)
    BCpk[:, 0:64, 0] = dtB[0::2]
    BCpk[:, 64:128, 0] = dtB[1::2]
    BCpk[:, 0:64, 1] = C[0::2]
    BCpk[:, 64:128, 1] = C[1::2]
    return Epk, ETpk, H0pk, H0Tpk, BCpk, I128


def _in_maps(A, B, C, log_dt):
    Epk, ETpk, H0pk, H0Tpk, BCpk, I128 = _host_pack(A, B, C, log_dt)
    maps = []
    for c in range(NCORES):
        s = slice(c * NT, (c + 1) * NT)
        maps.append({"Ein": Epk[s], "ETin": ETpk[s], "H0in": H0pk[s],
                     "H0Tin": H0Tpk[s], "BCin": BCpk[s], "Iin": I128})
    return maps


def kernel(A, B, C, log_dt, L):
    from concourse.bass_utils import run_bass_kernel_spmd

    if "nc" not in _cache:
        _cache["nc"] = _build_program(NT)
    nc = _cache["nc"]

    res = run_bass_kernel_spmd(nc, _in_maps(A, B, C, log_dt),
                               core_ids=list(range(NCORES)))
    K = np.empty((H, L), np.float32)
    for c in range(NCORES):
        K[c * CH_PER_CORE:(c + 1) * CH_PER_CORE] = (
            res.results[c]["Y"].reshape(CH_PER_CORE, L))
    return K
